# revision 39
# baseline (speedup 1.0000x reference)
"""Trainium2 Bass kernel: CausalParticleAttention (time-major, causal-skipped).

Problem: B=16 batches, N=16 particles, T=48 timesteps, C=512 channels,
H=8 heads (hd=64), attention over L=N*T=768 with per-head relative
time-position bias (T x T), relative particle-position bias (N x N) and a
causal mask over the time axis only; then output projection.

Sharding: pure data parallel over batch, 2 batches per NeuronCore x 8 cores.

Key layout choice: rows are TIME-MAJOR (r = t*NP + n). The causal mask
(t2 > t1 masked, particles all-visible) then makes S block-lower-triangular
at 128-row granularity (128 rows = 8 timesteps x 16 particles).

v2 structure (vs the phased baseline):
  - TIGHT triangular ranges: per (b,h) only the 2688 needed S columns are
    computed/exp'd/PV'd (jc chunk j covers i in [128*jc, 768)), packed into
    three [128,1024] PSUM tiles (2 banks each, pool bufs=2 -> 4 banks).
  - P and v are bf16 so narrow PV pieces run at 1 cycle/col (f32r would hit
    the 4x sub-256-width penalty).
  - All PSUM pools coexist (pj 2 banks + S 4 + yp 2 = 8): batch 1's
    projections interleave into batch 0's attention stream, so the PE fills
    the gaps that exp (Act) pacing would otherwise leave; batch 0's output
    projections hide inside batch 1's attention the same way.
  - Normalize chain: DVE reciprocal reads the PSUM rowsum directly; Pool
    copies y to SBUF, broadcasts the reciprocal and multiplies into yT.
"""

import sys

sys.path.insert(0, "/opt/trn_rl_repo")

import numpy as np

import concourse.bacc as bacc
import concourse.bass as bass
import concourse.mybir as mybir
import concourse.tile as tile
from concourse import bass_utils

F32 = mybir.dt.float32
F32R = mybir.dt.float32r
BF16 = mybir.dt.bfloat16
EXP = mybir.ActivationFunctionType.Exp

B_TOT = 16      # total batches
H = 8           # heads
T = 48          # timesteps
NP = 16         # particles
C = 512         # channels
HD = C // H     # 64 head dim
L = NP * T      # 768 sequence per batch
NCORES = 8
BPC = B_TOT // NCORES   # 2 batches per core
R = BPC * L             # 1536 rows per core
MASK = -1600.0          # pre-scale mask add: -200 * sqrt(hd)
SCALE = 0.125           # 1/sqrt(hd)

# S^T packing: three [128,1024] psum tiles per head (2 banks each).
# Entry: (jc, i0, i1, c0) -> S^T[j in jc-chunk, i in [i0,i1)] at cols
# c0:c0+(i1-i0). All pieces are bank-contained (banks at cols 0/512).
S_TILES = (
    (((0, 0, 512, 0), (0, 512, 768, 512), (4, 512, 768, 768)), 1024),
    (((1, 128, 640, 0), (1, 640, 768, 512), (3, 384, 768, 640)), 1024),
    (((2, 256, 768, 0), (5, 640, 768, 512)), 640),
)
# jc -> (tile_idx, col_base, i_lo)
PVSRC = {0: (0, 0, 0), 4: (0, 768, 512), 1: (1, 0, 128), 3: (1, 640, 384),
         2: (2, 0, 256), 5: (2, 512, 640)}
# PV write plan: (jc, i0, i1, start, stop); each yp element is started by
# jc0 and stopped by jc==region. Split A/B so QK tiles interleave between.
PV_PLAN_A = (
    (0, 0, 128, True, True), (0, 128, 512, True, False),
    (0, 512, 768, True, False),
    (1, 128, 256, False, True), (1, 256, 512, False, False),
    (1, 512, 768, False, False),
)
PV_PLAN_B = (
    (3, 384, 512, False, False), (3, 512, 768, False, False),
    (2, 256, 512, False, True), (2, 512, 768, False, False),
    (4, 512, 640, False, True), (4, 640, 768, False, False),
    (5, 640, 768, False, True),
)


def build_nc():
    nc = bacc.Bacc("TRN2", target_bir_lowering=False, debug=False)

    xt_d = nc.dram_tensor("xt", [BPC, 4, 128, L], BF16, kind="ExternalInput").ap()
    wq_d = nc.dram_tensor("wq", [4, 128, C], BF16, kind="ExternalInput").ap()
    wk_d = nc.dram_tensor("wk", [4, 128, C], BF16, kind="ExternalInput").ap()
    wv_d = nc.dram_tensor("wv", [4, 128, C], BF16, kind="ExternalInput").ap()
    wo_d = nc.dram_tensor("wo", [4, 128, C], F32R, kind="ExternalInput").ap()
    oh_d = nc.dram_tensor("onehot", [64, L], BF16, kind="ExternalInput").ap()
    kb_d = nc.dram_tensor("kbias", [H, 64, L], BF16, kind="ExternalInput").ap()
    out_d = nc.dram_tensor("out", [R, C], F32, kind="ExternalOutput").ap()

    with tile.TileContext(nc) as tc:
        _body(tc, xt_d, wq_d, wk_d, wv_d, wo_d, oh_d, kb_d, out_d)
    nc.compile()
    return nc


def _body(tc, xt_d, wq_d, wk_d, wv_d, wo_d, oh_d, kb_d, out_d):
    nc = tc.nc
    from contextlib import ExitStack

    with ExitStack() as ctx:
        const = ctx.enter_context(tc.tile_pool(name="const", bufs=1))
        persist = ctx.enter_context(tc.tile_pool(name="persist", bufs=1))

        ones_f32 = const.tile([128, 64], F32, name="ones_f32")
        nc.vector.memset(ones_f32, 1.0)
        zbias = const.tile([128, 1], F32, name="zbias")
        nc.vector.memset(zbias, 0.0)
        ones_bf16 = const.tile([128, 64], BF16, name="ones_bf16")
        nc.vector.memset(ones_bf16, 1.0)

        wq_sb = persist.tile([128, 4, C], BF16, name="wq_sb")
        wk_sb = persist.tile([128, 4, C], BF16, name="wk_sb")
        wv_sb = persist.tile([128, 4, C], BF16, name="wv_sb")
        wo_sb = persist.tile([128, 4, C], F32R, name="wo_sb")
        xT = [persist.tile([128, 4, L], BF16, name=f"xT{b}", tag=f"xT{b}")
              for b in range(BPC)]

        qa = {(b, h): persist.tile([128, L], BF16, name=f"qa{b}_{h}",
                                   tag=f"qa{b}_{h}")
              for b in range(BPC) for h in range(H)}
        ka = {(b, h): persist.tile([128, L], BF16, name=f"ka{b}_{h}",
                                   tag=f"ka{b}_{h}")
              for b in range(BPC) for h in range(H)}
        # vA col 64 is the ones column -> PV row 64 = softmax rowsum
        vA = [persist.tile([128, 6, H, HD + 1], BF16, name=f"vA{b}", tag=f"vA{b}")
              for b in range(BPC)]
        yT = {(b, c): persist.tile([128, L], F32R, name=f"yT{b}_{c}",
                                   tag=f"yT{b}_{c}")
              for b in range(BPC) for c in range(4)}
        for b in range(BPC):
            nc.vector.tensor_copy(
                out=vA[b][:, :, :, HD:HD + 1],
                in_=ones_f32[:, 0:48].rearrange("p (a h) -> p a h", a=6).unsqueeze(3))

        # single unified PSUM ring (6 banks) + yp (1.5 banks)
        s_ps = ctx.enter_context(tc.tile_pool(name="s_ps", bufs=3, space="PSUM"))
        y_ps = ctx.enter_context(tc.tile_pool(name="y_ps", bufs=1, space="PSUM"))
        pt_pool = ctx.enter_context(tc.tile_pool(name="pt_sb", bufs=20))
        ys_pool = ctx.enter_context(tc.tile_pool(name="ys_sb", bufs=6))
        rc_pool = ctx.enter_context(tc.tile_pool(name="rc_sb", bufs=4))
        bc_pool = ctx.enter_context(tc.tile_pool(name="bc_sb", bufs=4))
        fo_pool = ctx.enter_context(tc.tile_pool(name="fo_sb", bufs=4))

        # ---------------- startup DMAs (consumption order) ----------------
        for ci in range(4):
            nc.sync.dma_start(out=xT[0][:, ci, :], in_=xt_d[0, ci])
            nc.sync.dma_start(out=wq_sb[:, ci, :], in_=wq_d[ci])
        nc.sync.dma_start(out=wk_sb, in_=wk_d.rearrange("c p o -> p c o"))
        nc.sync.dma_start(out=qa[(0, 0)][64:128, :], in_=oh_d)
        for h in range(H):
            nc.sync.dma_start(out=ka[(0, h)][64:128, :], in_=kb_d[h])
        nc.sync.dma_start(out=wv_sb, in_=wv_d.rearrange("c p o -> p c o"))
        nc.sync.dma_start(out=xT[1], in_=xt_d[1].rearrange("c p i -> p c i"))
        nc.sync.dma_start(out=wo_sb, in_=wo_d.rearrange("c p o -> p c o"))
        for h in range(1, H):
            nc.gpsimd.tensor_copy(out=qa[(0, h)][64:128, :],
                                  in_=qa[(0, 0)][64:128, :])

        # ---------------- op emitters ----------------
        def qg(b, cc, w_sb, dst, hi_on_act, act_half=False, p_major=False):
            ps = s_ps.tile([128, 1024], F32, name="pjp", tag="sp")
            loops = [(p, ci) for p in ((0, 512), (512, 768)) for ci in range(4)] \
                if p_major else \
                [(p, ci) for ci in range(4) for p in ((0, 512), (512, 768))]
            for (p0, p1), ci in loops:
                nc.tensor.matmul(
                    ps[:, p0:p1],
                    lhsT=w_sb[:, ci, cc * 128:(cc + 1) * 128],
                    rhs=xT[b][:, ci, p0:p1],
                    start=(ci == 0), stop=(ci == 3))
            if act_half:
                nc.scalar.copy(out=dst[(b, 2 * cc)][0:64, 0:384],
                               in_=ps[0:64, 0:384])
                nc.vector.tensor_copy(out=dst[(b, 2 * cc)][0:64, 384:768],
                                      in_=ps[0:64, 384:768])
            elif hi_on_act:
                nc.scalar.copy(out=dst[(b, 2 * cc)][0:64, :], in_=ps[0:64, 0:768])
            else:
                nc.vector.tensor_copy(out=dst[(b, 2 * cc)][0:64, :],
                                      in_=ps[0:64, 0:768])
            nc.vector.tensor_copy(out=dst[(b, 2 * cc + 1)][0:64, :],
                                  in_=ps[64:128, 0:768])

        def qg_half(b, cc, w_sb, ci0, ci1, ps):
            for ci in (ci0, ci1):
                for p0, p1 in ((0, 512), (512, 768)):
                    nc.tensor.matmul(
                        ps[:, p0:p1],
                        lhsT=w_sb[:, ci, cc * 128:(cc + 1) * 128],
                        rhs=xT[b][:, ci, p0:p1],
                        start=(ci == ci0 and ci0 == 0),
                        stop=(ci == ci1 and ci1 == 3))

        def vg(b, l, on_act):
            ps = s_ps.tile([128, 1024], F32, name="vjp", tag="sp")
            for ci in range(4):
                nc.tensor.matmul(
                    ps[:, 0:C], lhsT=xT[b][:, ci, l * 128:(l + 1) * 128],
                    rhs=wv_sb[:, ci, :],
                    start=(ci == 0), stop=(ci == 3))
            src = ps[:, 0:C].rearrange("p (h d) -> p h d", h=H)
            if on_act:
                nc.scalar.copy(out=vA[b][:, l, :, 0:HD], in_=src)
            else:
                nc.vector.tensor_copy(out=vA[b][:, l, :, 0:HD], in_=src)

        def fin(b, ic, fo_act=False):
            fp = s_ps.tile([128, C], F32, name="fp", tag="sp")
            for c4 in range(4):
                nc.tensor.matmul(
                    fp[:, 0:C],
                    lhsT=yT[(b, c4)][:, ic * 128:(ic + 1) * 128],
                    rhs=wo_sb[:, c4, :],
                    start=(c4 == 0), stop=(c4 == 3))
            fo = fo_pool.tile([128, C], F32, name="fo", tag="fo")
            if fo_act:
                nc.scalar.copy(out=fo, in_=fp[:, 0:C])
            else:
                nc.vector.tensor_copy(out=fo, in_=fp[:, 0:C])
            nc.sync.dma_start(
                out=out_d[b * L + ic * 128:b * L + (ic + 1) * 128, :],
                in_=fo)

        def fan(h):
            nc.sync.dma_start(out=qa[(1, h)][64:128, :],
                              in_=qa[(0, 0)][64:128, :])
            nc.sync.dma_start(out=ka[(1, h)][64:128, :],
                              in_=ka[(0, h)][64:128, :])

        # ---------------- interleaved schedule ----------------
        pts = {}
        yps = {}

        def QG(b, cc, hi_on_act=False, act_half=False, p_major=False):
            qg(b, cc, wq_sb, qa, hi_on_act, act_half, p_major)

        def KG(b, cc, hi_on_act=False, act_half=False, p_major=False):
            qg(b, cc, wk_sb, ka, hi_on_act, act_half, p_major)

        def Q(b, h, ti):
            if ti == 0:
                pts[(b, h)] = []
            pieces, width = S_TILES[ti]
            sp = s_ps.tile([128, 1024], F32, name="sp", tag="sp")
            for jc, i0, i1, c0 in pieces:
                nc.tensor.matmul(
                    sp[:, c0:c0 + (i1 - i0)],
                    lhsT=ka[(b, h)][:, jc * 128:(jc + 1) * 128],
                    rhs=qa[(b, h)][:, i0:i1],
                    start=True, stop=True)
            pt = pt_pool.tile([128, width], BF16, name="pt", tag="pt",
                              padded_shape=[128, 1024])
            nc.scalar.activation(out=pt, in_=sp[:, 0:width], func=EXP,
                                 bias=zbias, scale=SCALE)
            pts[(b, h)].append(pt)

        def PV(b, h, plan):
            if (b, h) not in yps:
                if (b, h) == (1, 7):
                    yps[(b, h)] = s_ps.tile([128, 1024], F32, name="yp7",
                                            tag="sp")
                else:
                    yps[(b, h)] = y_ps.tile([128, L], F32, name="yp", tag="yp")
            yp = yps[(b, h)]
            p = pts[(b, h)]
            for jc, i0, i1, st, sp_ in plan:
                ti, base, i_lo = PVSRC[jc]
                nc.tensor.matmul(
                    yp[0:HD + 1, i0:i1],
                    lhsT=vA[b][:, jc, h, :],
                    rhs=p[ti][:, base + i0 - i_lo:base + i1 - i_lo],
                    start=st, stop=sp_)

        def A(b, h):
            PV(b, h, PV_PLAN_A)

        def B(b, h):
            PV(b, h, PV_PLAN_B)

        NL = {}

        def N(b, h, last=False):
            del pts[(b, h)]
            yp = yps.pop((b, h))
            cc, par = divmod(h, 2)
            if last:
                rcp = rc_pool.tile([1, L], BF16, name="rcpb", tag="rcp")
                with nc.allow_low_precision(reason="bf16 recip feeds PE bcast"):
                    nc.vector.reciprocal(out=rcp, in_=yp[HD:HD + 1, 0:L])
                ys = ys_pool.tile([HD, L], F32, name="ysl", tag="ys")
                nc.vector.tensor_copy(out=ys, in_=yp[0:HD, 0:L])
                NL["rcp"], NL["ys"], NL["cc"], NL["par"], NL["b"] = \
                    rcp, ys, cc, par, b
                return
            ys = ys_pool.tile([HD + 1, L], F32, name="ys", tag="ys")
            nc.vector.tensor_copy(out=ys, in_=yp[0:HD + 1, 0:L])
            rcp = rc_pool.tile([1, L], F32, name="rcp", tag="rcp")
            nc.vector.reciprocal(out=rcp, in_=ys[HD:HD + 1, :])
            bcs = bc_pool.tile([64, L], F32, name="bcs", tag="bcs")
            nc.gpsimd.partition_broadcast(out_ap=bcs, in_ap=rcp)
            nc.gpsimd.tensor_tensor(
                yT[(b, cc)][par * 64:par * 64 + 64, :],
                ys[0:HD, :], bcs, mybir.AluOpType.mult)

        def N2():
            rcp, ys, cc, par, b = NL["rcp"], NL["ys"], NL["cc"], NL["par"], NL["b"]
            bcp = y_ps.tile([64, L], F32, name="bcp", tag="yp")
            for p0, p1 in ((0, 512), (512, L)):
                nc.tensor.matmul(bcp[:, p0:p1],
                                 lhsT=ones_bf16[0:1, 0:HD],
                                 rhs=rcp[:, p0:p1],
                                 start=True, stop=True)
            nc.vector.tensor_tensor(
                yT[(b, cc)][par * 64:par * 64 + 64, :],
                ys, bcp, mybir.AluOpType.mult)

        def THIRD(b, ic, pool=None, tag=None):
            fp = (pool or s_ps).tile([128, C], F32, name="fp", tag=tag or "sp")
            for c4 in range(3):
                nc.tensor.matmul(
                    fp[:, 0:C],
                    lhsT=yT[(b, c4)][:, ic * 128:(ic + 1) * 128],
                    rhs=wo_sb[:, c4, :],
                    start=(c4 == 0), stop=False)
            return fp

        def CLOSE(b, ic, fp, fo_act):
            nc.tensor.matmul(
                fp[:, 0:C],
                lhsT=yT[(b, 3)][:, ic * 128:(ic + 1) * 128],
                rhs=wo_sb[:, 3, :],
                start=False, stop=True)
            fo = fo_pool.tile([128, C], F32, name="fo", tag="fo")
            if fo_act:
                nc.scalar.copy(out=fo, in_=fp[:, 0:C])
            else:
                nc.vector.tensor_copy(out=fo, in_=fp[:, 0:C])
            nc.sync.dma_start(
                out=out_d[b * L + ic * 128:b * L + (ic + 1) * 128, :],
                in_=fo)

        # --- alpha: b0 q,k proj, QK heads 0..3 skewed one block behind ---
        QG(0, 0, act_half=True); KG(0, 0, act_half=True)
        for cc in range(1, 4):
            Q(0, cc - 1, 0); QG(0, cc, act_half=True); Q(0, cc - 1, 1)
            KG(0, cc, act_half=True); Q(0, cc - 1, 2)
        Q(0, 3, 0); vg(0, 0, True); Q(0, 3, 1); vg(0, 1, False)
        Q(0, 3, 2); vg(0, 2, True)

        # --- beta: b0 heads 4..7 + v drain + b1 proj start ---
        Q(0, 4, 0); vg(0, 3, False); Q(0, 4, 1); vg(0, 4, True)
        Q(0, 4, 2); vg(0, 5, False); fan(0)
        A(0, 0); Q(0, 5, 0); B(0, 0); N(0, 0); Q(0, 5, 1)
        QG(1, 0, True); Q(0, 5, 2); fan(1)
        A(0, 1); Q(0, 6, 0); B(0, 1); N(0, 1); Q(0, 6, 1)
        KG(1, 0, True); Q(0, 6, 2); fan(2)
        A(0, 2); Q(0, 7, 0); B(0, 2); N(0, 2); Q(0, 7, 1)
        QG(1, 1, True); Q(0, 7, 2); fan(3)

        # --- gamma: b0 PV drain + b1 proj + b1 QK heads 0..7 ---
        A(0, 3); Q(1, 0, 0); B(0, 3); N(0, 3); Q(1, 0, 1); KG(1, 1, True)
        A(0, 4); Q(1, 0, 2); B(0, 4); N(0, 4); QG(1, 2, True)
        A(0, 5); Q(1, 1, 0); B(0, 5); N(0, 5); Q(1, 1, 1); KG(1, 2, True)
        A(0, 6); Q(1, 1, 2); B(0, 6); N(0, 6); QG(1, 3, True); fan(4)
        A(0, 7); Q(1, 2, 0); B(0, 7); N(0, 7); Q(1, 2, 1); KG(1, 3, True)
        Q(1, 2, 2); vg(1, 0, False); fan(5)
        Q(1, 3, 0); vg(1, 1, False); Q(1, 3, 1); vg(1, 2, False)
        Q(1, 3, 2); fan(6)
        Q(1, 4, 0); vg(1, 3, False); Q(1, 4, 1); vg(1, 4, False)
        Q(1, 4, 2); fan(7)
        Q(1, 5, 0); vg(1, 5, False); Q(1, 5, 1)
        A(1, 0); Q(1, 5, 2); B(1, 0); N(1, 0)
        A(1, 1); Q(1, 6, 0); B(1, 1); N(1, 1); Q(1, 6, 1)
        fin(0, 0, fo_act=True); Q(1, 6, 2)
        A(1, 2); Q(1, 7, 0); B(1, 2); N(1, 2); Q(1, 7, 1)
        fin(0, 1, fo_act=False); Q(1, 7, 2)

        # --- delta: b1 PV drain + b0 finals + output thirds ---
        A(1, 3); fin(0, 2, fo_act=True); B(1, 3); N(1, 3)
        fin(0, 3, fo_act=False)
        A(1, 4); fin(0, 4, fo_act=True); B(1, 4); N(1, 4)
        A(1, 5); fin(0, 5, fo_act=False); B(1, 5); N(1, 5)
        A(1, 6); fp0 = THIRD(1, 0); B(1, 6); N(1, 6)
        A(1, 7); fp1 = THIRD(1, 1); B(1, 7); N(1, 7, last=True)

        # --- epsilon tail ---
        N2()
        CLOSE(1, 0, fp0, fo_act=True)
        fp2 = THIRD(1, 2)
        CLOSE(1, 1, fp1, fo_act=False)
        CLOSE(1, 2, fp2, fo_act=True)
        fin(1, 3, fo_act=False)
        fin(1, 4, fo_act=True)
        fin(1, 5, fo_act=False)


def host_tables(rel_pos_bias, particle_rel_pos_bias):
    """onehot [64, L] and kbias [H, 64, L] fp32 host constants (time-major)."""
    rel_pos_bias = np.asarray(rel_pos_bias, np.float32)        # [2T-1, H]
    particle_rel_pos_bias = np.asarray(particle_rel_pos_bias, np.float32)  # [2NP-1, H]
    idx = np.arange(L)
    it, ip = idx // NP, idx % NP          # t1(i), n1(i)  (time-major rows)
    onehot = np.zeros((64, L), np.float32)
    onehot[it, idx] = 1.0
    onehot[T + ip, idx] = 1.0

    jt, jn = idx // NP, idx % NP          # t2(j), n2(j)
    t1 = np.arange(T)[:, None]
    bt = rel_pos_bias[(jt[None, :] - t1) + (T - 1)]            # [T, L, H]
    ktop = 8.0 * np.transpose(bt, (2, 0, 1))                   # [H, T, L]
    ktop = ktop + np.where(jt[None, :] > t1, MASK, 0.0)[None]
    n1 = np.arange(NP)[:, None]
    bp = particle_rel_pos_bias[(jn[None, :] - n1) + (NP - 1)]  # [NP, L, H]
    kbot = 8.0 * np.transpose(bp, (2, 0, 1))                   # [H, NP, L]
    kbias = np.concatenate([ktop, kbot], axis=1).astype(np.float32)
    return onehot, np.ascontiguousarray(kbias)


def make_in_maps(x, Wq, Wk, Wv, Wo, rel_pos_bias, particle_rel_pos_bias):
    import ml_dtypes
    bf16 = ml_dtypes.bfloat16
    x = np.ascontiguousarray(np.asarray(x, np.float32))
    ws = [np.ascontiguousarray(np.asarray(w, np.float32).reshape(4, 128, C))
          for w in (Wq, Wk, Wv, Wo)]
    onehot, kbias = host_tables(rel_pos_bias, particle_rel_pos_bias)
    # time-major rows r = t*NP + n, then pre-transposed to [C, rows] chunks
    xs = x.reshape(B_TOT, NP, T, C).transpose(0, 2, 1, 3)      # (B, T, NP, C)
    in_maps = []
    for c in range(NCORES):
        xc = xs[BPC * c:BPC * (c + 1)].reshape(BPC, L, C)
        xt = np.ascontiguousarray(
            xc.transpose(0, 2, 1).reshape(BPC, 4, 128, L))     # (b, ci, p, i)
        in_maps.append({
            "xt": xt.astype(bf16), "wq": ws[0].astype(bf16),
            "wk": ws[1].astype(bf16), "wv": ws[2].astype(bf16), "wo": ws[3],
            "onehot": onehot.astype(bf16), "kbias": kbias.astype(bf16),
        })
    return in_maps


def unshard_core(out_core):
    """[R, C] time-major rows -> (BPC, NP, T, C)."""
    return out_core.reshape(BPC, T, NP, C).transpose(0, 2, 1, 3)


_NC_CACHE = None


def _get_nc():
    global _NC_CACHE
    if _NC_CACHE is None:
        _NC_CACHE = build_nc()
    return _NC_CACHE


def kernel(x, Wq, Wk, Wv, Wo, rel_pos_bias, particle_rel_pos_bias):
    in_maps = make_in_maps(x, Wq, Wk, Wv, Wo, rel_pos_bias, particle_rel_pos_bias)
    res = bass_utils.run_bass_kernel_spmd(
        _get_nc(), in_maps, core_ids=list(range(NCORES)))
    outs = [unshard_core(res.results[c]["out"]) for c in range(NCORES)]
    return np.ascontiguousarray(np.concatenate(outs, axis=0))
